# revision 8
# baseline (speedup 1.0000x reference)
"""Trainium2 Bass kernel for ClassicalSelfAttention.

  out = softmax((X @ R) @ (X @ E).T / sqrt(D)) @ X,  X: (8192, 1024) fp32

Sharding: sequence-parallel over 8 NeuronCores. Core i owns queries
[i*1024, (i+1)*1024).

Uses scores = X (R E^T) X^T: the raw X^T blocks are AllGathered (no
compute dependency, starts at t~0) in TWO rounds — pairwise exchange
(i <-> i^1), then groups-of-4 over evens/odds, which lands blocks in
identity order — so the b=1 key block (the pair partner) is available
after the small round-1 exchange. Meanwhile each core computes
H = R^T X_i^T then G^T = E H = (X_i R E^T)^T locally; scores for key
block b contract G^T against the gathered X^T block directly. Block 0
uses the X_i^T tile already in SBUF.

Score matmuls run in float32r (~13-bit mantissa, full PE rate) — needed
because softmax gaps are O(1) while scores are O(1000). The probability
matrix P and the PV matmul run in bf16 (P in [0,1], X replicated in bf16
by the host), which halves weight-load time and PV DMA traffic.
"""
import numpy as np
import ml_dtypes

import concourse.bass as bass_mod
import concourse.bacc as bacc
import concourse.mybir as mybir
from concourse import tile
from concourse.bass_utils import run_bass_kernel_spmd
from concourse.masks import make_identity

DT = mybir.dt
F32 = DT.float32
F32R = DT.float32r
BF16 = DT.bfloat16
ALU = mybir.AluOpType
ACTF = mybir.ActivationFunctionType

S, D, NCORES = 8192, 1024, 8
SL = S // NCORES          # 1024 queries per core
P = 128                   # partitions
DC = D // P               # 8 contraction chunks
MC = SL // P              # 8 query chunks per core
TB = 1024                 # key block size
NB = S // TB              # 8 key blocks
SCALE = 1.0 / 32.0        # 1/sqrt(D)
NEG_BIG = -1.0e30


def build_program(num_devices=NCORES):
    nc = bacc.Bacc("TRN2", target_bir_lowering=False, debug=False,
                   num_devices=num_devices)

    xt = nc.declare_dram_parameter("xt", [D, SL], F32R, isOutput=False)
    r_p = nc.declare_dram_parameter("r", [D, D], F32R, isOutput=False)
    et_p = nc.declare_dram_parameter("et", [D, D], F32R, isOutput=False)
    xb_p = nc.declare_dram_parameter("xb16", [S, D], BF16, isOutput=False)
    out_p = nc.declare_dram_parameter("out", [SL, D], F32, isOutput=True)

    with tile.TileContext(nc) as tc:
        with (
            tc.tile_pool(name="persist", bufs=1) as pers,
            tc.tile_pool(name="dram", bufs=1, space="DRAM") as dram,
            tc.tile_pool(name="kt", bufs=2) as ktp,
        ):
            xt_own = dram.tile([D, SL], F32R, name="xt_own")
            r1out = dram.tile([2 * D, SL], F32R, name="r1out")
            xtall = dram.tile([NCORES * D, SL], F32R, name="xtall")

            g = pers.tile([P, DC * SL], F32R, tag="g")        # G^T, [w | q]
            oacc = pers.tile([P, MC * D], F32, tag="oacc")    # O accum per m
            ident32 = pers.tile([P, P], F32, tag="ident32")
            ident = pers.tile([P, P], BF16, tag="ident")
            mst = [[pers.tile([P, 1], F32, tag=f"mst{m}_{j}", name=f"mst{m}_{j}")
                    for j in range(2)] for m in range(MC)]
            sig = [pers.tile([P, 1], F32, tag=f"sig{m}", name=f"sig{m}")
                   for m in range(MC)]

            # ---- stage X_i^T to internal DRAM, then 2-round gather -------
            # round 1: pairwise (2k, 2k+1) -> r1out = [blk_even; blk_odd]
            # round 2: evens/odds groups of 4, each member contributing its
            # pair's 2 blocks -> xtall lands in identity (global) order.
            for k in range(DC):
                nc.scalar.dma_start(xt_own[k * P:(k + 1) * P, :],
                                    xt[k * P:(k + 1) * P, :])
            nc.gpsimd.collective_compute(
                "AllGather",
                ALU.bypass,
                replica_groups=[[0, 1], [2, 3], [4, 5], [6, 7]],
                ins=[xt_own.opt()],
                outs=[r1out.opt()],
            )
            nc.gpsimd.collective_compute(
                "AllGather",
                ALU.bypass,
                replica_groups=[[0, 2, 4, 6], [1, 3, 5, 7]],
                ins=[r1out.opt()],
                outs=[xtall.opt()],
            )

            make_identity(nc, ident32[:])
            nc.vector.tensor_copy(ident[:], ident32[:])
            nc.gpsimd.memset(oacc[:], 0.0)
            for m in range(MC):
                nc.gpsimd.memset(mst[m][0][:], NEG_BIG)
                nc.gpsimd.memset(sig[m][:], 0.0)

            # xt_sb doubles as the b=0 key tile, so it lives in the kt pool
            xt_sb = ktp.tile([P, DC * SL], F32R, tag="kt", name="xt_sb")
            for k in range(DC):
                nc.sync.dma_start(
                    xt_sb[:, k * SL:(k + 1) * SL],
                    xt[k * P:(k + 1) * P, :])

            # ---------------- Phase A: G^T = (X_i R E^T)^T ---------------
            with (
                tc.tile_pool(name="pa", bufs=1) as pa,
                tc.tile_pool(name="pa_ps", bufs=2, space="PSUM") as pa_ps,
            ):
                r_sb = pa.tile([P, DC * D], F32R, tag="re")    # R  [d | c]
                h_sb = pa.tile([P, DC * SL], F32R, tag="h")    # H [c | q]
                for k in range(DC):
                    nc.sync.dma_start(
                        r_sb[:, k * D:(k + 1) * D],
                        r_p[k * P:(k + 1) * P, :])

                # H = R^T @ X_i^T  [c, q]
                for o in range(DC):
                    ps = pa_ps.tile([P, SL], F32, tag="proj")
                    for h in range(SL // 512):
                        for k in range(DC):
                            nc.tensor.matmul(
                                ps[:, h * 512:(h + 1) * 512],
                                r_sb[:, k * D + o * P: k * D + (o + 1) * P],
                                xt_sb[:, k * SL + h * 512:
                                      k * SL + (h + 1) * 512],
                                start=(k == 0), stop=(k == DC - 1),
                            )
                    nc.vector.tensor_copy(h_sb[:, o * SL:(o + 1) * SL], ps[:])

                # E^T loads into R's slot once H is done with it
                et_sb = pa.tile([P, DC * D], F32R, tag="re", name="et_sb")
                for k in range(DC):
                    nc.scalar.dma_start(
                        et_sb[:, k * D:(k + 1) * D],
                        et_p[k * P:(k + 1) * P, :])

                # G^T = E H  [w, q]   (lhsT = E^T chunks)
                for o in range(DC):
                    ps = pa_ps.tile([P, SL], F32, tag="proj")
                    for h in range(SL // 512):
                        for k in range(DC):
                            nc.tensor.matmul(
                                ps[:, h * 512:(h + 1) * 512],
                                et_sb[:, k * D + o * P: k * D + (o + 1) * P],
                                h_sb[:, k * SL + h * 512:
                                     k * SL + (h + 1) * 512],
                                start=(k == 0), stop=(k == DC - 1),
                            )
                    nc.vector.tensor_copy(g[:, o * SL:(o + 1) * SL], ps[:])

            # ---------------- Phase B: blocked attention -----------------
            # Software-pipelined by one m-step: PE runs transposes+PV of the
            # previous (b, m) while DVE/ACT compute stats+exp of the current.
            with (
                tc.tile_pool(name="xb", bufs=2) as xbp,
                tc.tile_pool(name="ph", bufs=4) as php,
                tc.tile_pool(name="pt", bufs=2) as ptp,
                tc.tile_pool(name="ofin", bufs=2) as ofp,
                tc.tile_pool(name="stats", bufs=6) as stp,
                tc.tile_pool(name="s_ps", bufs=4, space="PSUM") as sps,
                tc.tile_pool(name="t_ps", bufs=2, space="PSUM") as tps,
                tc.tile_pool(name="o_ps", bufs=1, space="PSUM") as ops,
            ):
                def flush_pe(pend):
                    ph, alpha, m, b, xb = pend
                    o_part = ops.tile([P, D], F32, tag="opart", name="o_part")
                    tp = tps.tile([P, TB], BF16, tag="tp", name="tp")
                    for cc in range(8):
                        nc.tensor.transpose(
                            tp[:, cc * P:(cc + 1) * P],
                            ph[:, cc * P:(cc + 1) * P],
                            ident[:],
                        )
                    pt = ptp.tile([P, TB], BF16, tag="pt", name="pt")
                    nc.vector.tensor_copy(pt[:], tp[:])
                    for cc in range(8):
                        for h in range(D // 512):
                            nc.tensor.matmul(
                                o_part[:, h * 512:(h + 1) * 512],
                                pt[:, cc * P:(cc + 1) * P],
                                xb[:, cc * D + h * 512:
                                   cc * D + (h + 1) * 512],
                                start=(cc == 0), stop=(cc == 7),
                            )
                    return o_part

                def flush_dve(pend, o_part):
                    ph, alpha, m, b, xb = pend
                    nc.vector.scalar_tensor_tensor(
                        oacc[:, m * D:(m + 1) * D],
                        oacc[:, m * D:(m + 1) * D],
                        alpha[:], o_part[:],
                        op0=ALU.mult, op1=ALU.add)
                    if b == NB - 1:
                        # finalize this m: divide by softmax sum and store
                        rcp = stp.tile([P, 1], F32, tag="rcp", name="rcp")
                        nc.vector.reciprocal(rcp[:], sig[m][:])
                        of = ofp.tile([P, D], F32, tag="ofin", name="ofin")
                        nc.vector.tensor_scalar_mul(
                            of[:], oacc[:, m * D:(m + 1) * D], rcp[:])
                        nc.sync.dma_start(out_p[m * P:(m + 1) * P, :], of[:])

                pending = []
                pid = nc.sync.partition_id()
                for b in range(NB):
                    # block order: own block, then the round-1 partner
                    # (pid^1), then the remaining blocks ring-style.
                    if b == 0:
                        kt = xt_sb
                        gexp = None
                    elif b == 1:
                        kt = ktp.tile([P, DC * TB], F32R, tag="kt", name="kt")
                        kt_src = r1out[bass_mod.ds(((pid + 1) % 2) * D, D), :]
                        nc.sync.dma_start(
                            kt.rearrange("p (k c) -> p k c", k=DC),
                            kt_src.rearrange("(k p) c -> p k c", p=P))
                        gexp = (pid + 1 + 6 * (pid % 2)) % 8
                    else:
                        kt = ktp.tile([P, DC * TB], F32R, tag="kt", name="kt")
                        gexp = (pid + b + 7 * (pid % 2)) % 8
                        kt_src = xtall[bass_mod.ds(gexp * D, D), :]
                        nc.sync.dma_start(
                            kt.rearrange("p (k c) -> p k c", k=DC),
                            kt_src.rearrange("(k p) c -> p k c", p=P))
                    xb = xbp.tile([P, (TB // P) * D], BF16, tag="xb",
                                  name="xb")
                    xb_off = (pid * TB if b == 0
                              else ((pid + 1 + 6 * (pid % 2)) % 8) * TB
                              if b == 1
                              else ((pid + b + 7 * (pid % 2)) % 8) * TB)
                    nc.sync.dma_start(
                        xb.rearrange("p (k c) -> p k c", k=TB // P),
                        xb_p[bass_mod.ds(xb_off, TB), :]
                        .rearrange("(k p) c -> p k c", p=P))

                    for m in range(MC):
                        # scores in two 512-halves (h-outer) so stats/exp of
                        # half 0 overlap the matmuls of half 1
                        sh_ = [sps.tile([P, 512], F32, tag="s", name="s")
                               for _ in range(2)]
                        mqh = [stp.tile([P, 1], F32, tag=f"mq{h}",
                                        name=f"mq{h}") for h in range(2)]
                        for h in range(2):
                            for k in range(DC):
                                lhsT = g[:, k * SL + m * P:
                                         k * SL + (m + 1) * P]
                                nc.tensor.matmul(
                                    sh_[h][:],
                                    lhsT,
                                    kt[:, k * TB + h * 512:
                                       k * TB + (h + 1) * 512],
                                    start=(k == 0), stop=(k == DC - 1),
                                )
                            nc.vector.reduce_max(mqh[h][:], sh_[h][:],
                                                 axis=mybir.AxisListType.X)

                        # online softmax stats; mst ping-pongs on b parity
                        m_old = mst[m][b % 2]
                        mnew = mst[m][(b + 1) % 2]
                        mq = stp.tile([P, 1], F32, tag="mq", name="mq")
                        nc.vector.tensor_max(mq[:], mqh[0][:], mqh[1][:])
                        nc.vector.tensor_max(mnew[:], m_old[:], mq[:])
                        nbias = stp.tile([P, 1], F32, tag="nbias", name="nbias")
                        nc.scalar.mul(nbias[:], mnew[:], -SCALE)
                        # alpha = exp(s*m_old + nbias) = exp((m_old - mnew)/32)
                        alpha = stp.tile([P, 1], F32, tag="alpha", name="alpha")
                        nc.scalar.activation(alpha[:], m_old[:], ACTF.Exp,
                                             bias=nbias[:], scale=SCALE)

                        # phat = exp(s/32 - mnew/32), per half; sums into sq
                        ph = php.tile([P, TB], BF16, tag="ph", name="ph")
                        sqh = [stp.tile([P, 1], F32, tag=f"sq{h}",
                                        name=f"sq{h}") for h in range(2)]
                        for h in range(2):
                            nc.scalar.activation(ph[:, h * 512:(h + 1) * 512],
                                                 sh_[h][:], ACTF.Exp,
                                                 bias=nbias[:], scale=SCALE,
                                                 accum_out=sqh[h][:])
                        sq = stp.tile([P, 1], F32, tag="sq", name="sq")
                        nc.vector.tensor_add(sq[:], sqh[0][:], sqh[1][:])
                        nc.vector.scalar_tensor_tensor(
                            sig[m][:], sig[m][:], alpha[:], sq[:],
                            op0=ALU.mult, op1=ALU.add)

                        pending.append((ph, alpha, m, b, xb))
                        if len(pending) > 2:
                            pend_fl = pending.pop(0)
                            flush_dve(pend_fl, flush_pe(pend_fl))
                for pend in pending:
                    flush_dve(pend, flush_pe(pend))

    nc.compile()
    return nc


_PROGRAM = None


def _get_program():
    global _PROGRAM
    if _PROGRAM is None:
        _PROGRAM = build_program()
    return _PROGRAM


def kernel(inputs, rotation_params, entangle_params, _trace=False):
    X = np.ascontiguousarray(np.asarray(inputs, dtype=np.float32))
    R = np.ascontiguousarray(np.asarray(rotation_params, dtype=np.float32))
    E = np.ascontiguousarray(np.asarray(entangle_params, dtype=np.float32))
    assert X.shape == (S, D) and R.shape == (D, D) and E.shape == (D, D)

    XT = np.ascontiguousarray(X.T)
    ET = np.ascontiguousarray(E.T)
    X16 = X.astype(ml_dtypes.bfloat16)
    in_maps = []
    for i in range(NCORES):
        in_maps.append({
            "xt": np.ascontiguousarray(XT[:, i * SL:(i + 1) * SL]),
            "r": R,
            "et": ET,
            "xb16": X16,
        })

    nc = _get_program()
    res = run_bass_kernel_spmd(nc, in_maps, list(range(NCORES)),
                               trace=_trace)
    out = np.concatenate([res.results[i]["out"] for i in range(NCORES)],
                         axis=0)
    if _trace:
        return out, res
    return out


# revision 9
# speedup vs baseline: 1.3298x; 1.3298x over previous
"""Trainium2 Bass kernel for ClassicalSelfAttention.

  out = softmax((X @ R) @ (X @ E).T / sqrt(D)) @ X,  X: (8192, 1024) fp32

Sharding: sequence-parallel over 8 NeuronCores. Core i owns queries
[i*1024, (i+1)*1024).

Uses scores = X (R E^T) X^T: the raw X^T blocks are AllGathered (no
compute dependency, starts at t~0) as TWO sequential 8-core collectives,
one per 512-column half of X_i^T. Attention runs over 16 half-blocks of
512 keys: both halves of the own block first (from SBUF), then the seven
gathered h0 half-blocks (needs only gather 1), then the seven h1
half-blocks — so each gather is fully hidden behind compute. Meanwhile
each core computes H = R^T X_i^T then G^T = E H = (X_i R E^T)^T locally;
scores contract G^T against gathered raw X^T, no per-block key
projection.

Score matmuls run in float32r (~13-bit mantissa, full PE rate) — needed
because softmax gaps are O(1) while scores are O(1000). The probability
matrix P and the PV matmul run in bf16 (P in [0,1], X replicated in bf16
by the host), which halves weight-load time and PV DMA traffic.
"""
import numpy as np
import ml_dtypes

import concourse.bass as bass_mod
import concourse.bacc as bacc
import concourse.mybir as mybir
from concourse import tile
from concourse.bass_utils import run_bass_kernel_spmd
from concourse.masks import make_identity

DT = mybir.dt
F32 = DT.float32
F32R = DT.float32r
BF16 = DT.bfloat16
ALU = mybir.AluOpType
ACTF = mybir.ActivationFunctionType

S, D, NCORES = 8192, 1024, 8
SL = S // NCORES          # 1024 queries per core
P = 128                   # partitions
DC = D // P               # 8 contraction chunks
MC = SL // P              # 8 query chunks per core
HB = 512                  # key half-block size
NSTEP = 16                # 16 half-blocks of 512 keys
SCALE = 1.0 / 32.0        # 1/sqrt(D)
NEG_BIG = -1.0e30


def build_program(num_devices=NCORES):
    nc = bacc.Bacc("TRN2", target_bir_lowering=False, debug=False,
                   num_devices=num_devices)

    xt = nc.declare_dram_parameter("xt", [D, SL], F32R, isOutput=False)
    r_p = nc.declare_dram_parameter("r", [D, D], F32R, isOutput=False)
    et_p = nc.declare_dram_parameter("et", [D, D], F32R, isOutput=False)
    xb_p = nc.declare_dram_parameter("xb16", [S, D], BF16, isOutput=False)
    out_p = nc.declare_dram_parameter("out", [SL, D], F32, isOutput=True)

    with tile.TileContext(nc) as tc:
        with (
            tc.tile_pool(name="persist", bufs=1) as pers,
            tc.tile_pool(name="dram", bufs=1, space="DRAM") as dram,
            tc.tile_pool(name="xts", bufs=1) as xtsp,
        ):
            xt_h = [dram.tile([D, HB], F32R, name=f"xt_h{h}")
                    for h in range(2)]
            xtall_h = [dram.tile([NCORES * D, HB], F32R, addr_space="Shared",
                                 name=f"xtall_h{h}") for h in range(2)]

            g = pers.tile([P, DC * SL], F32R, tag="g")        # G^T, [w | q]
            oacc = pers.tile([P, MC * D], F32, tag="oacc")    # O accum per m
            ident32 = pers.tile([P, P], F32, tag="ident32")
            ident = pers.tile([P, P], BF16, tag="ident")
            mst = [[pers.tile([P, 1], F32, tag=f"mst{m}_{j}", name=f"mst{m}_{j}")
                    for j in range(2)] for m in range(MC)]
            sig = [pers.tile([P, 1], F32, tag=f"sig{m}", name=f"sig{m}")
                   for m in range(MC)]

            # ---- stage X_i^T halves to internal DRAM, gather each half ---
            for k in range(DC):
                for h in range(2):
                    nc.scalar.dma_start(
                        xt_h[h][k * P:(k + 1) * P, :],
                        xt[k * P:(k + 1) * P, h * HB:(h + 1) * HB])
            for h in range(2):
                nc.gpsimd.collective_compute(
                    "AllGather",
                    ALU.bypass,
                    replica_groups=[list(range(NCORES))],
                    ins=[xt_h[h].opt()],
                    outs=[xtall_h[h].opt()],
                )

            make_identity(nc, ident32[:])
            nc.vector.tensor_copy(ident[:], ident32[:])
            nc.gpsimd.memset(oacc[:], 0.0)
            for m in range(MC):
                nc.gpsimd.memset(mst[m][0][:], NEG_BIG)
                nc.gpsimd.memset(sig[m][:], 0.0)

            # X_i^T in SBUF: feeds H, and is the key tile for steps 0/1
            xt_sb = xtsp.tile([P, DC * SL], F32R, tag="xt", name="xt_sb")
            for k in range(DC):
                nc.sync.dma_start(
                    xt_sb[:, k * SL:(k + 1) * SL],
                    xt[k * P:(k + 1) * P, :])

            # ---------------- Phase A: G^T = (X_i R E^T)^T ---------------
            with (
                tc.tile_pool(name="pa", bufs=1) as pa,
                tc.tile_pool(name="pa_ps", bufs=2, space="PSUM") as pa_ps,
            ):
                r_sb = pa.tile([P, DC * D], F32R, tag="r")     # R  [d | c]
                et_sb = pa.tile([P, DC * D], F32R, tag="et")   # E^T [c | w]
                h_sb = pa.tile([P, DC * SL], F32R, tag="h")    # H [c | q]
                for k in range(DC):
                    nc.sync.dma_start(
                        r_sb[:, k * D:(k + 1) * D],
                        r_p[k * P:(k + 1) * P, :])
                    nc.scalar.dma_start(
                        et_sb[:, k * D:(k + 1) * D],
                        et_p[k * P:(k + 1) * P, :])

                # H = R^T @ X_i^T  [c, q]
                for o in range(DC):
                    ps = pa_ps.tile([P, SL], F32, tag="proj")
                    for h in range(SL // 512):
                        for k in range(DC):
                            nc.tensor.matmul(
                                ps[:, h * 512:(h + 1) * 512],
                                r_sb[:, k * D + o * P: k * D + (o + 1) * P],
                                xt_sb[:, k * SL + h * 512:
                                      k * SL + (h + 1) * 512],
                                start=(k == 0), stop=(k == DC - 1),
                            )
                    nc.vector.tensor_copy(h_sb[:, o * SL:(o + 1) * SL], ps[:])

                # G^T = E H  [w, q]   (lhsT = E^T chunks)
                for o in range(DC):
                    ps = pa_ps.tile([P, SL], F32, tag="proj")
                    for h in range(SL // 512):
                        for k in range(DC):
                            nc.tensor.matmul(
                                ps[:, h * 512:(h + 1) * 512],
                                et_sb[:, k * D + o * P: k * D + (o + 1) * P],
                                h_sb[:, k * SL + h * 512:
                                     k * SL + (h + 1) * 512],
                                start=(k == 0), stop=(k == DC - 1),
                            )
                    nc.vector.tensor_copy(g[:, o * SL:(o + 1) * SL], ps[:])

            # ---------------- Phase B: blocked attention -----------------
            # 16 steps of 512 keys: [own h0, own h1, ring h0 x7, ring h1 x7]
            # Software-pipelined: PE runs transposes+PV of step j-2 while
            # DVE/ACT compute stats+exp of step j.
            with (
                tc.tile_pool(name="kt", bufs=3) as ktp,
                tc.tile_pool(name="xb", bufs=3) as xbp,
                tc.tile_pool(name="ph", bufs=4) as php,
                tc.tile_pool(name="pt", bufs=2) as ptp,
                tc.tile_pool(name="ofin", bufs=2) as ofp,
                tc.tile_pool(name="stats", bufs=6) as stp,
                tc.tile_pool(name="s_ps", bufs=4, space="PSUM") as sps,
                tc.tile_pool(name="t_ps", bufs=2, space="PSUM") as tps,
                tc.tile_pool(name="o_ps", bufs=1, space="PSUM") as ops,
            ):
                def flush_pe(pend):
                    ph, alpha, m, stepi, xb = pend
                    o_part = ops.tile([P, D], F32, tag="opart", name="o_part")
                    tp = tps.tile([P, HB], BF16, tag="tp", name="tp")
                    for cc in range(4):
                        nc.tensor.transpose(
                            tp[:, cc * P:(cc + 1) * P],
                            ph[:, cc * P:(cc + 1) * P],
                            ident[:],
                        )
                    pt = ptp.tile([P, HB], BF16, tag="pt", name="pt")
                    nc.vector.tensor_copy(pt[:], tp[:])
                    for cc in range(4):
                        for h in range(D // 512):
                            nc.tensor.matmul(
                                o_part[:, h * 512:(h + 1) * 512],
                                pt[:, cc * P:(cc + 1) * P],
                                xb[:, cc * D + h * 512:
                                   cc * D + (h + 1) * 512],
                                start=(cc == 0), stop=(cc == 3),
                            )
                    return o_part

                def flush_dve(pend, o_part):
                    ph, alpha, m, stepi, xb = pend
                    nc.vector.scalar_tensor_tensor(
                        oacc[:, m * D:(m + 1) * D],
                        oacc[:, m * D:(m + 1) * D],
                        alpha[:], o_part[:],
                        op0=ALU.mult, op1=ALU.add)
                    if stepi == NSTEP - 1:
                        # finalize this m: divide by softmax sum and store
                        rcp = stp.tile([P, 1], F32, tag="rcp", name="rcp")
                        nc.vector.reciprocal(rcp[:], sig[m][:])
                        of = ofp.tile([P, D], F32, tag="ofin", name="ofin")
                        nc.vector.tensor_scalar_mul(
                            of[:], oacc[:, m * D:(m + 1) * D], rcp[:])
                        nc.sync.dma_start(out_p[m * P:(m + 1) * P, :], of[:])

                pending = []
                pid = nc.sync.partition_id()
                # step schedule: own block from SBUF first, then ring h0
                # (gather 1 only), then ring h1 (gather 2).
                sched = ([("own", 0), ("own", 1)]
                         + [("ring", 0)] * (NCORES - 1)
                         + [("ring", 1)] * (NCORES - 1))
                ring_pos = {0: 0, 1: 0}
                for stepi, (src, hb) in enumerate(sched):
                    if src == "own":
                        kt = None
                        xb_off = pid * SL + hb * HB
                    else:
                        ring_pos[hb] += 1
                        gb = (pid + ring_pos[hb]) % NCORES
                        kt = ktp.tile([P, DC * HB], F32R, tag="kt", name="kt")
                        kt_src = xtall_h[hb][bass_mod.ds(gb * D, D), :]
                        nc.sync.dma_start(
                            kt.rearrange("p (k c) -> p k c", k=DC),
                            kt_src.rearrange("(k p) c -> p k c", p=P))
                        xb_off = gb * SL + hb * HB
                    xb = xbp.tile([P, (HB // P) * D], BF16, tag="xb",
                                  name="xb")
                    nc.sync.dma_start(
                        xb.rearrange("p (k c) -> p k c", k=HB // P),
                        xb_p[bass_mod.ds(xb_off, HB), :]
                        .rearrange("(k p) c -> p k c", p=P))

                    for m in range(MC):
                        sh = sps.tile([P, HB], F32, tag="s", name="s")
                        for k in range(DC):
                            lhsT = g[:, k * SL + m * P: k * SL + (m + 1) * P]
                            if src == "own":
                                rhs = xt_sb[:, k * SL + hb * HB:
                                            k * SL + hb * HB + HB]
                            else:
                                rhs = kt[:, k * HB:(k + 1) * HB]
                            nc.tensor.matmul(
                                sh[:], lhsT, rhs,
                                start=(k == 0), stop=(k == DC - 1),
                            )
                        mq = stp.tile([P, 1], F32, tag="mq", name="mq")
                        nc.vector.reduce_max(mq[:], sh[:],
                                             axis=mybir.AxisListType.X)

                        # online softmax stats; mst ping-pongs on step parity
                        m_old = mst[m][stepi % 2]
                        mnew = mst[m][(stepi + 1) % 2]
                        nc.vector.tensor_max(mnew[:], m_old[:], mq[:])
                        nbias = stp.tile([P, 1], F32, tag="nbias", name="nbias")
                        nc.scalar.mul(nbias[:], mnew[:], -SCALE)
                        # alpha = exp(s*m_old + nbias) = exp((m_old - mnew)/32)
                        alpha = stp.tile([P, 1], F32, tag="alpha", name="alpha")
                        nc.scalar.activation(alpha[:], m_old[:], ACTF.Exp,
                                             bias=nbias[:], scale=SCALE)

                        # phat = exp(s/32 - mnew/32); row sums into sq
                        ph = php.tile([P, HB], BF16, tag="ph", name="ph")
                        sq = stp.tile([P, 1], F32, tag="sq", name="sq")
                        nc.scalar.activation(ph[:], sh[:], ACTF.Exp,
                                             bias=nbias[:], scale=SCALE,
                                             accum_out=sq[:])
                        nc.vector.scalar_tensor_tensor(
                            sig[m][:], sig[m][:], alpha[:], sq[:],
                            op0=ALU.mult, op1=ALU.add)

                        pending.append((ph, alpha, m, stepi, xb))
                        if len(pending) > 2:
                            pend_fl = pending.pop(0)
                            flush_dve(pend_fl, flush_pe(pend_fl))
                for pend in pending:
                    flush_dve(pend, flush_pe(pend))

    nc.compile()
    return nc


_PROGRAM = None


def _get_program():
    global _PROGRAM
    if _PROGRAM is None:
        _PROGRAM = build_program()
    return _PROGRAM


def kernel(inputs, rotation_params, entangle_params, _trace=False):
    X = np.ascontiguousarray(np.asarray(inputs, dtype=np.float32))
    R = np.ascontiguousarray(np.asarray(rotation_params, dtype=np.float32))
    E = np.ascontiguousarray(np.asarray(entangle_params, dtype=np.float32))
    assert X.shape == (S, D) and R.shape == (D, D) and E.shape == (D, D)

    XT = np.ascontiguousarray(X.T)
    ET = np.ascontiguousarray(E.T)
    X16 = X.astype(ml_dtypes.bfloat16)
    in_maps = []
    for i in range(NCORES):
        in_maps.append({
            "xt": np.ascontiguousarray(XT[:, i * SL:(i + 1) * SL]),
            "r": R,
            "et": ET,
            "xb16": X16,
        })

    nc = _get_program()
    res = run_bass_kernel_spmd(nc, in_maps, list(range(NCORES)),
                               trace=_trace)
    out = np.concatenate([res.results[i]["out"] for i in range(NCORES)],
                         axis=0)
    if _trace:
        return out, res
    return out


# revision 11
# speedup vs baseline: 1.3311x; 1.0009x over previous
"""Trainium2 Bass kernel for ClassicalSelfAttention.

  out = softmax((X @ R) @ (X @ E).T / sqrt(D)) @ X,  X: (8192, 1024) fp32

Sharding: sequence-parallel over 8 NeuronCores. Core i owns queries
[i*1024, (i+1)*1024).

Uses scores = X (R E^T) X^T: the raw X^T blocks are AllGathered (no
compute dependency, starts at t~0) as TWO sequential 8-core collectives,
one per 512-column half of X_i^T. Attention runs over 16 half-blocks of
512 keys: both halves of the own block first (from SBUF), then the seven
gathered h0 half-blocks (needs only gather 1), then the seven h1
half-blocks — so each gather is fully hidden behind compute. Meanwhile
each core computes H = R^T X_i^T then G^T = E H = (X_i R E^T)^T locally;
scores contract G^T against gathered raw X^T, no per-block key
projection.

Score matmuls run in float32r (~13-bit mantissa, full PE rate) — needed
because softmax gaps are O(1) while scores are O(1000). The probability
matrix P and the PV matmul run in bf16 (P in [0,1], X replicated in bf16
by the host), which halves weight-load time and PV DMA traffic.
"""
import numpy as np
import ml_dtypes

import concourse.bass as bass_mod
import concourse.bacc as bacc
import concourse.mybir as mybir
from concourse import tile
from concourse.bass_utils import run_bass_kernel_spmd
from concourse.masks import make_identity

DT = mybir.dt
F32 = DT.float32
F32R = DT.float32r
BF16 = DT.bfloat16
ALU = mybir.AluOpType
ACTF = mybir.ActivationFunctionType

S, D, NCORES = 8192, 1024, 8
SL = S // NCORES          # 1024 queries per core
P = 128                   # partitions
DC = D // P               # 8 contraction chunks
MC = SL // P              # 8 query chunks per core
HB = 512                  # key half-block size
NSTEP = 16                # 16 half-blocks of 512 keys
SCALE = 1.0 / 32.0        # 1/sqrt(D)
NEG_BIG = -1.0e30


def build_program(num_devices=NCORES):
    nc = bacc.Bacc("TRN2", target_bir_lowering=False, debug=False,
                   num_devices=num_devices)

    xt = nc.declare_dram_parameter("xt", [D, SL], F32R, isOutput=False)
    r_p = nc.declare_dram_parameter("r", [D, D], F32R, isOutput=False)
    et_p = nc.declare_dram_parameter("et", [D, D], F32R, isOutput=False)
    xb_p = nc.declare_dram_parameter("xb16", [S, D], BF16, isOutput=False)
    out_p = nc.declare_dram_parameter("out", [SL, D], F32, isOutput=True)

    with tile.TileContext(nc) as tc:
        with (
            tc.tile_pool(name="persist", bufs=1) as pers,
            tc.tile_pool(name="dram", bufs=1, space="DRAM") as dram,
            tc.tile_pool(name="xts", bufs=1) as xtsp,
        ):
            xt_h = [dram.tile([D, HB], F32R, name=f"xt_h{h}")
                    for h in range(2)]
            xtall_h = [dram.tile([NCORES * D, HB], F32R, addr_space="Shared",
                                 name=f"xtall_h{h}") for h in range(2)]

            g = pers.tile([P, DC * SL], F32R, tag="g")        # G^T, [w | q]
            oacc = pers.tile([P, MC * D], F32, tag="oacc")    # O accum per m
            ident32 = pers.tile([P, P], F32, tag="ident32")
            ident = pers.tile([P, P], BF16, tag="ident")
            mst = [[pers.tile([P, 1], F32, tag=f"mst{m}_{j}", name=f"mst{m}_{j}")
                    for j in range(2)] for m in range(MC)]
            sig = [pers.tile([P, 1], F32, tag=f"sig{m}", name=f"sig{m}")
                   for m in range(MC)]

            # ---- stage X_i^T halves to internal DRAM, gather each half ---
            for k in range(DC):
                for h in range(2):
                    nc.gpsimd.dma_start(
                        xt_h[h][k * P:(k + 1) * P, :],
                        xt[k * P:(k + 1) * P, h * HB:(h + 1) * HB])
            for h in range(2):
                nc.gpsimd.collective_compute(
                    "AllGather",
                    ALU.bypass,
                    replica_groups=[list(range(NCORES))],
                    ins=[xt_h[h].opt()],
                    outs=[xtall_h[h].opt()],
                )

            make_identity(nc, ident32[:])
            nc.vector.tensor_copy(ident[:], ident32[:])
            nc.gpsimd.memset(oacc[:], 0.0)
            for m in range(MC):
                nc.gpsimd.memset(mst[m][0][:], NEG_BIG)
                nc.gpsimd.memset(sig[m][:], 0.0)

            # X_i^T in SBUF: feeds H, and is the key tile for steps 0/1
            xt_sb = xtsp.tile([P, DC * SL], F32R, tag="xt", name="xt_sb")

            # ---------------- Phase A: G^T = (X_i R E^T)^T ---------------
            with (
                tc.tile_pool(name="pa", bufs=1) as pa,
                tc.tile_pool(name="pa_ps", bufs=2, space="PSUM") as pa_ps,
            ):
                r_sb = pa.tile([P, DC * D], F32R, tag="r")     # R  [d | c]
                et_sb = pa.tile([P, DC * D], F32R, tag="et")   # E^T [c | w]
                h_sb = pa.tile([P, DC * SL], F32R, tag="h")    # H [c | q]
                for k in range(DC):
                    nc.sync.dma_start(
                        r_sb[:, k * D:(k + 1) * D],
                        r_p[k * P:(k + 1) * P, :])
                    nc.sync.dma_start(
                        xt_sb[:, k * SL:(k + 1) * SL],
                        xt[k * P:(k + 1) * P, :])
                    nc.scalar.dma_start(
                        et_sb[:, k * D:(k + 1) * D],
                        et_p[k * P:(k + 1) * P, :])

                # H = R^T @ X_i^T  [c, q]
                for o in range(DC):
                    ps = pa_ps.tile([P, SL], F32, tag="proj")
                    for h in range(SL // 512):
                        for k in range(DC):
                            nc.tensor.matmul(
                                ps[:, h * 512:(h + 1) * 512],
                                r_sb[:, k * D + o * P: k * D + (o + 1) * P],
                                xt_sb[:, k * SL + h * 512:
                                      k * SL + (h + 1) * 512],
                                start=(k == 0), stop=(k == DC - 1),
                            )
                    nc.vector.tensor_copy(h_sb[:, o * SL:(o + 1) * SL], ps[:])

                # G^T = E H  [w, q]   (lhsT = E^T chunks)
                for o in range(DC):
                    ps = pa_ps.tile([P, SL], F32, tag="proj")
                    for h in range(SL // 512):
                        for k in range(DC):
                            nc.tensor.matmul(
                                ps[:, h * 512:(h + 1) * 512],
                                et_sb[:, k * D + o * P: k * D + (o + 1) * P],
                                h_sb[:, k * SL + h * 512:
                                     k * SL + (h + 1) * 512],
                                start=(k == 0), stop=(k == DC - 1),
                            )
                    nc.vector.tensor_copy(g[:, o * SL:(o + 1) * SL], ps[:])

            # ---------------- Phase B: blocked attention -----------------
            # 16 steps of 512 keys: [own h0, own h1, ring h0 x7, ring h1 x7]
            # Software-pipelined: PE runs transposes+PV of step j-2 while
            # DVE/ACT compute stats+exp of step j.
            with (
                tc.tile_pool(name="kt", bufs=3) as ktp,
                tc.tile_pool(name="xb", bufs=3) as xbp,
                tc.tile_pool(name="ph", bufs=4) as php,
                tc.tile_pool(name="pt", bufs=2) as ptp,
                tc.tile_pool(name="ofin", bufs=2) as ofp,
                tc.tile_pool(name="stats", bufs=6) as stp,
                tc.tile_pool(name="s_ps", bufs=4, space="PSUM") as sps,
                tc.tile_pool(name="t_ps", bufs=2, space="PSUM") as tps,
                tc.tile_pool(name="o_ps", bufs=1, space="PSUM") as ops,
            ):
                def flush_pe(pend):
                    ph, alpha, m, stepi, xb = pend
                    o_part = ops.tile([P, D], F32, tag="opart", name="o_part")
                    tp = tps.tile([P, HB], BF16, tag="tp", name="tp")
                    for cc in range(4):
                        nc.tensor.transpose(
                            tp[:, cc * P:(cc + 1) * P],
                            ph[:, cc * P:(cc + 1) * P],
                            ident[:],
                        )
                    pt = ptp.tile([P, HB], BF16, tag="pt", name="pt")
                    nc.vector.tensor_copy(pt[:], tp[:])
                    for cc in range(4):
                        for h in range(D // 512):
                            nc.tensor.matmul(
                                o_part[:, h * 512:(h + 1) * 512],
                                pt[:, cc * P:(cc + 1) * P],
                                xb[:, cc * D + h * 512:
                                   cc * D + (h + 1) * 512],
                                start=(cc == 0), stop=(cc == 3),
                            )
                    return o_part

                def flush_dve(pend, o_part):
                    ph, alpha, m, stepi, xb = pend
                    nc.vector.scalar_tensor_tensor(
                        oacc[:, m * D:(m + 1) * D],
                        oacc[:, m * D:(m + 1) * D],
                        alpha[:], o_part[:],
                        op0=ALU.mult, op1=ALU.add)
                    if stepi == NSTEP - 1:
                        # finalize this m: divide by softmax sum and store
                        rcp = stp.tile([P, 1], F32, tag="rcp", name="rcp")
                        nc.vector.reciprocal(rcp[:], sig[m][:])
                        of = ofp.tile([P, D], F32, tag="ofin", name="ofin")
                        nc.vector.tensor_scalar_mul(
                            of[:], oacc[:, m * D:(m + 1) * D], rcp[:])
                        nc.sync.dma_start(out_p[m * P:(m + 1) * P, :], of[:])

                pending = []
                pid = nc.sync.partition_id()
                # step schedule: own block from SBUF first, then ring h0
                # (gather 1 only), then ring h1 (gather 2).
                sched = ([("own", 0), ("own", 1)]
                         + [("ring", 0)] * (NCORES - 1)
                         + [("ring", 1)] * (NCORES - 1))
                ring_pos = {0: 0, 1: 0}
                for stepi, (src, hb) in enumerate(sched):
                    if src == "own":
                        kt = None
                        xb_off = pid * SL + hb * HB
                    else:
                        ring_pos[hb] += 1
                        gb = (pid + ring_pos[hb]) % NCORES
                        kt = ktp.tile([P, DC * HB], F32R, tag="kt", name="kt")
                        kt_src = xtall_h[hb][bass_mod.ds(gb * D, D), :]
                        nc.sync.dma_start(
                            kt.rearrange("p (k c) -> p k c", k=DC),
                            kt_src.rearrange("(k p) c -> p k c", p=P))
                        xb_off = gb * SL + hb * HB
                    xb = xbp.tile([P, (HB // P) * D], BF16, tag="xb",
                                  name="xb")
                    nc.sync.dma_start(
                        xb.rearrange("p (k c) -> p k c", k=HB // P),
                        xb_p[bass_mod.ds(xb_off, HB), :]
                        .rearrange("(k p) c -> p k c", p=P))

                    for m in range(MC):
                        sh = sps.tile([P, HB], F32, tag="s", name="s")
                        for k in range(DC):
                            lhsT = g[:, k * SL + m * P: k * SL + (m + 1) * P]
                            if src == "own":
                                rhs = xt_sb[:, k * SL + hb * HB:
                                            k * SL + hb * HB + HB]
                            else:
                                rhs = kt[:, k * HB:(k + 1) * HB]
                            nc.tensor.matmul(
                                sh[:], lhsT, rhs,
                                start=(k == 0), stop=(k == DC - 1),
                            )
                        mq = stp.tile([P, 1], F32, tag="mq", name="mq")
                        nc.vector.reduce_max(mq[:], sh[:],
                                             axis=mybir.AxisListType.X)

                        # online softmax stats; mst ping-pongs on step parity
                        m_old = mst[m][stepi % 2]
                        mnew = mst[m][(stepi + 1) % 2]
                        nc.vector.tensor_max(mnew[:], m_old[:], mq[:])
                        nbias = stp.tile([P, 1], F32, tag="nbias", name="nbias")
                        nc.scalar.mul(nbias[:], mnew[:], -SCALE)
                        # alpha = exp(s*m_old + nbias) = exp((m_old - mnew)/32)
                        alpha = stp.tile([P, 1], F32, tag="alpha", name="alpha")
                        nc.scalar.activation(alpha[:], m_old[:], ACTF.Exp,
                                             bias=nbias[:], scale=SCALE)

                        # phat = exp(s/32 - mnew/32); row sums into sq
                        ph = php.tile([P, HB], BF16, tag="ph", name="ph")
                        sq = stp.tile([P, 1], F32, tag="sq", name="sq")
                        nc.scalar.activation(ph[:], sh[:], ACTF.Exp,
                                             bias=nbias[:], scale=SCALE,
                                             accum_out=sq[:])
                        nc.vector.scalar_tensor_tensor(
                            sig[m][:], sig[m][:], alpha[:], sq[:],
                            op0=ALU.mult, op1=ALU.add)

                        pending.append((ph, alpha, m, stepi, xb))
                        if len(pending) > 2:
                            pend_fl = pending.pop(0)
                            flush_dve(pend_fl, flush_pe(pend_fl))
                for pend in pending:
                    flush_dve(pend, flush_pe(pend))

    nc.compile()
    return nc


_PROGRAM = None


def _get_program():
    global _PROGRAM
    if _PROGRAM is None:
        _PROGRAM = build_program()
    return _PROGRAM


def kernel(inputs, rotation_params, entangle_params, _trace=False):
    X = np.ascontiguousarray(np.asarray(inputs, dtype=np.float32))
    R = np.ascontiguousarray(np.asarray(rotation_params, dtype=np.float32))
    E = np.ascontiguousarray(np.asarray(entangle_params, dtype=np.float32))
    assert X.shape == (S, D) and R.shape == (D, D) and E.shape == (D, D)

    XT = np.ascontiguousarray(X.T)
    ET = np.ascontiguousarray(E.T)
    X16 = X.astype(ml_dtypes.bfloat16)
    in_maps = []
    for i in range(NCORES):
        in_maps.append({
            "xt": np.ascontiguousarray(XT[:, i * SL:(i + 1) * SL]),
            "r": R,
            "et": ET,
            "xb16": X16,
        })

    nc = _get_program()
    res = run_bass_kernel_spmd(nc, in_maps, list(range(NCORES)),
                               trace=_trace)
    out = np.concatenate([res.results[i]["out"] for i in range(NCORES)],
                         axis=0)
    if _trace:
        return out, res
    return out
